# revision 6
# baseline (speedup 1.0000x reference)
"""NOTEARS loss kernel for Trainium2 (8 NeuronCores, Bass/Tile).

Math: with W_m = W with zeroed diagonal, A = I - W_m^T, G = X^T X:
    ||X - X W_m^T||_F^2 = tr(A^T G A)
so the only T-sized work is the Gram accumulation G = sum_i X_i^T X_i,
data-parallel over 8 cores.  X is quantized to fp8 e4m3 on the host
(16 MB/core of DMA; shifts the loss by ~8e-4 relative vs the 2e-2
tolerance) and each core streams its shard at the HBM roofline
(~350 GB/s) into DoubleRow matmuls (two 128-row contraction subtiles
per instruction, 2 rows/cycle) accumulating G_i in f32 PSUM.  The
device outputs G_i [128,128]; everything W-sized -- A, the h(W) power
series, AL and L1 terms -- is combined on the host in float64.
"""

import numpy as np
import ml_dtypes

import concourse.bacc as bacc
import concourse.mybir as mybir
from concourse import tile
from concourse.bass_utils import run_bass_kernel_spmd

D = 128
T_TRUE = 1_000_000
N_CORES = 8
CHUNKS_PER_CORE = 978            # even: 489 DoubleRow pairs, 1472 zero-pad rows
ROWS_PER_CORE = CHUNKS_PER_CORE * D   # 125184
TILE_CHUNKS = 64                 # 64 chunks = [128, 8192] fp8 = 1 MiB per DMA

LAMBDA1 = 0.01
ALPHA_LAG = 0.5
RHO = 1.0
N_TERMS = 10
F32 = mybir.dt.float32
FP8 = mybir.dt.float8e4
DR = mybir.MatmulPerfMode.DoubleRow


def _build(chunks_per_core=CHUNKS_PER_CORE, tile_chunks=TILE_CHUNKS,
           stream_repeats=1, xbufs=8):
    rows_per_core = chunks_per_core * D
    full_tiles = chunks_per_core // tile_chunks
    tail_chunks = chunks_per_core - full_tiles * tile_chunks
    assert tile_chunks % 2 == 0 and tail_chunks % 2 == 0
    nc = bacc.Bacc("TRN2", target_bir_lowering=False, debug=False)
    xs = nc.dram_tensor("xs", [rows_per_core, D], FP8, kind="ExternalInput")
    g_out = nc.dram_tensor("g", [D, D], F32, kind="ExternalOutput")

    with tile.TileContext(nc) as tc:
        with (
            tc.tile_pool(name="xpool", bufs=xbufs) as xpool,
            tc.tile_pool(name="cpool", bufs=1) as cpool,
            tc.tile_pool(name="gpsum", bufs=1, space="PSUM") as gpsum_pool,
        ):
            # Gram accumulation is invariant to row ordering, so assign each
            # partition a CONTIGUOUS block of tile_chunks rows: each DMA then
            # reads tile_chunks*128 contiguous bytes per partition.  DoubleRow
            # contracts two 128-row subtiles per matmul (2 rows/cycle).
            g_ps = gpsum_pool.tile([D, D], F32)
            first = True
            for rep in range(stream_repeats):
                last_rep = rep == stream_repeats - 1
                if full_tiles > 0:
                    vf = (
                        xs.ap()[: full_tiles * tile_chunks * D, :]
                        .rearrange("(t p q) d -> t p q d", p=D, q=tile_chunks)
                    )
                    for t in range(full_tiles):
                        xt = xpool.tile([D, tile_chunks, D], FP8)
                        nc.sync.dma_start(xt[:], vf[t])
                        for j in range(0, tile_chunks, 2):
                            nc.tensor.matmul(
                                g_ps[:], xt[:, j : j + 2, :], xt[:, j : j + 2, :],
                                start=first, stop=False, perf_mode=DR,
                            )
                            first = False
                if tail_chunks > 0:
                    base_row = full_tiles * tile_chunks * D
                    vt = (
                        xs.ap()[base_row : base_row + tail_chunks * D, :]
                        .rearrange("(p q) d -> p q d", p=D, q=tail_chunks)
                    )
                    xtail = xpool.tile([D, tail_chunks, D], FP8)
                    nc.sync.dma_start(xtail[:], vt)
                    for j in range(0, tail_chunks, 2):
                        nc.tensor.matmul(
                            g_ps[:],
                            xtail[:, j : j + 2, :],
                            xtail[:, j : j + 2, :],
                            start=False,
                            stop=(last_rep and j == tail_chunks - 2),
                            perf_mode=DR,
                        )

            g_sb = cpool.tile([D, D], F32)
            nc.vector.tensor_copy(g_sb[:], g_ps[:])
            nc.sync.dma_start(g_out.ap(), g_sb[:])

    nc.compile()
    return nc


_NC = None


def _get_nc():
    global _NC
    if _NC is None:
        _NC = _build()
    return _NC


def _shard_inputs(X, W=None):
    # W is not a device input (loss combine is on host); accepted for
    # signature compatibility with test harnesses.
    X = np.ascontiguousarray(np.asarray(X, dtype=np.float32))
    Xp = np.empty((ROWS_PER_CORE * N_CORES, D), dtype=ml_dtypes.float8_e4m3)
    Xp[:T_TRUE] = X.astype(ml_dtypes.float8_e4m3)
    Xp[T_TRUE:] = 0  # zero-pad rows contribute nothing to G
    return [
        {"xs": Xp[i * ROWS_PER_CORE : (i + 1) * ROWS_PER_CORE]}
        for i in range(N_CORES)
    ], Xp


def _host_combine(G, W):
    """f64 loss combine from the summed Gram matrix."""
    W = np.asarray(W, dtype=np.float64)
    Wm = W * (1.0 - np.eye(D))
    A = np.eye(D) - Wm.T
    loss = 0.5 * np.einsum("ij,ij->", A, G @ A) / T_TRUE
    WW = Wm * Wm
    total, power, factorial = 0.0, WW.copy(), 1.0
    for k in range(1, N_TERMS):
        factorial *= k
        total += np.trace(power) / factorial
        if k < N_TERMS - 1:
            power = power @ WW
    h = total
    al_term = ALPHA_LAG * h + 0.5 * RHO * h * h
    l1 = LAMBDA1 * np.sum(np.abs(Wm))
    return loss + al_term + l1


def kernel(X, W):
    nc = _get_nc()
    in_maps, _xp = _shard_inputs(X)
    res = run_bass_kernel_spmd(nc, in_maps, core_ids=list(range(N_CORES)))
    G = np.sum(np.float64([r["g"] for r in res.results]), axis=0)
    return np.float32(_host_combine(G, W))
